# revision 20
# baseline (speedup 1.0000x reference)
"""GCNConv on 8 Trainium2 NeuronCores (Bass/Tile).

out = segsum_r( ew * (nodes @ W * rsqrt(deg_s)*rsqrt(deg_r))[senders] )  with self loops.

Two SPMD launches:
  L1 (node-sharded): per-node degrees (padded-grid reduce), scale, y = (X@W)*scale.
  L2 (receiver-sharded): per-edge dma_gather of y rows (A/B table split for int16
     indices), ew-weighted one-hot S built on DVE, segment-sum via PE matmul
     (S^T @ msgs accumulated in PSUM per 128-receiver tile).
Host does index/layout work only (sorting, chunking, padding); all FLOPs on device.
"""
import sys
sys.path.insert(0, '/opt/trn_rl_repo')
import numpy as np

N_NODES = 50000
D = 128
NCORES = 8
P = 128
SPLIT = 32768           # rows in gather table A; rest in B
GSIZE = 4               # receiver tiles per psum group
NQ = 4                  # SWDGE queues for gather overlap (queue q -> Q7 pair q)
MAXCHUNK = 14           # max 128-idx chunks per dma_gather call (queue interleave)


def _ceil(a, b):
    return (a + b - 1) // b


def _build(senders, receivers, edges, n_nodes, ncores, split):
    """Host-side index preprocessing. Returns per-core input dicts + metadata."""
    nt = _ceil(n_nodes, P * ncores) * ncores   # total tiles, multiple of ncores
    npad = nt * P
    tpc = nt // ncores
    e_w = np.concatenate([edges[:, 0], np.ones(n_nodes, edges.dtype)])
    cs = np.concatenate([senders, np.arange(n_nodes, dtype=np.int64)])
    cr = np.concatenate([receivers.astype(np.int64), np.arange(n_nodes, dtype=np.int64)])
    cs = cs.astype(np.int64)

    # ---- degree grids (launch 1), node-sharded by npad/ncores ----
    shard = npad // ncores
    deg_r_cnt = np.bincount(cr, minlength=npad).astype(np.int64)
    deg_s_cnt = np.bincount(cs, minlength=npad).astype(np.int64)
    padw = max(int(deg_r_cnt.max()), int(deg_s_cnt.max()))
    padw = _ceil(max(padw, 4), 4) * 4
    order_r = np.argsort(cr, kind='stable')
    order_s = np.argsort(cs, kind='stable')

    def grid(order, key, cnt):
        g = np.zeros((npad, padw), np.float32)
        pos = np.concatenate([[0], np.cumsum(cnt)])[:-1]
        off = np.arange(len(key)) - pos[key[order]]
        g[key[order], off] = e_w[order]
        return g

    grid_r = grid(order_r, cr, deg_r_cnt)
    grid_s = grid(order_s, cs, deg_s_cnt)
    cnts = (deg_r_cnt.astype(np.float32), deg_s_cnt.astype(np.float32))

    # ---- receiver-sharded chunk structure (launch 2) ----
    # self-loop block excluded: its contribution is exactly +y[r], added at
    # psum flush from a contiguous read instead of 50k gathered rows.
    n_real = len(senders)
    cs = cs[:n_real]
    cr = cr[:n_real]
    e_w2 = e_w[:n_real]
    tile_of = cr >> 7
    isA = cs < split
    # per (tile, ab): edge index lists
    by_tile = [[None, None] for _ in range(nt)]
    idx_sorted = np.argsort(tile_of * 2 + (~isA).astype(np.int64), kind='stable')
    key = tile_of * 2 + (~isA).astype(np.int64)
    bounds = np.searchsorted(key[idx_sorted], np.arange(2 * nt + 1))
    for t in range(nt):
        by_tile[t][0] = idx_sorted[bounds[2 * t]:bounds[2 * t + 1]]
        by_tile[t][1] = idx_sorted[bounds[2 * t + 1]:bounds[2 * t + 2]]

    # balance tiles across cores: sort by chunk count, deal 8 at a time so the
    # per-local-index max across cores ~= mean (minimizes pad chunks)
    ca_t = np.array([max(_ceil(len(by_tile[t][0]), P), 1) for t in range(nt)])
    cb_t = np.array([_ceil(len(by_tile[t][1]), P) for t in range(nt)])
    rank = np.argsort(-(ca_t + cb_t), kind='stable')
    tile_map = np.zeros((ncores, tpc), np.int64)
    for r, t in enumerate(rank):
        tile_map[r % ncores, r // ncores] = t
    cpa = np.zeros(tpc, np.int64)
    cpb = np.zeros(tpc, np.int64)
    for j in range(tpc):
        for k in range(ncores):
            t = int(tile_map[k, j])
            cpa[j] = max(cpa[j], ca_t[t])
            cpb[j] = max(cpb[j], cb_t[t])

    # build per-core streams; chunk sequence = groups of GSIZE tiles:
    # [A-chunks of g tiles][B-chunks of g tiles]
    groups = [list(range(g, min(g + GSIZE, tpc))) for g in range(0, tpc, GSIZE)]
    runs = []   # (ab, [local tiles], [chunks per tile]) per group, compile-time
    for g in groups:
        runs.append((0, g, [int(cpa[j]) for j in g]))
        if sum(int(cpb[j]) for j in g):
            runs.append((1, g, [int(cpb[j]) for j in g]))

    per_core = []
    for k in range(ncores):
        idxs = [[], []]
        rls = [[], []]
        ews = [[], []]
        nreg = [[], []]   # per-run valid count
        for ab, g, cps in runs:
            cnt = 0
            for j, nch in zip(g, cps):
                t = int(tile_map[k, j])
                el = by_tile[t][ab]
                need = nch * P
                ii = np.full(need, -1, np.int64)
                rr = np.zeros(need, np.float32)
                ee = np.zeros(need, np.float32)
                ii[:len(el)] = cs[el] - (split if ab else 0)
                rr[:len(el)] = (cr[el] - (t << 7)).astype(np.float32)
                ee[:len(el)] = e_w2[el]
                cnt += len(el)
                idxs[ab].append(ii)
                rls[ab].append(rr)
                ews[ab].append(ee)
            nreg[ab].append(cnt)

        def pack_idx(chunks):
            s = np.concatenate(chunks) if chunks else np.zeros(0, np.int64)
            w = s.reshape(-1, 16).T.astype(np.int16)          # [16, L/16]
            return np.tile(w, (8, 1))                          # [128, L/16]

        def pack_col(chunks):
            s = np.concatenate(chunks) if chunks else np.zeros(0, np.float32)
            return np.ascontiguousarray(s.reshape(-1, P).T)    # [128, C]

        per_core.append(dict(
            ia=pack_idx(idxs[0]), ib=pack_idx(idxs[1]) if idxs[1] else np.zeros((128, 8), np.int16),
            ra=pack_col(rls[0]), rb=pack_col(rls[1]) if rls[1] else np.zeros((128, 1), np.float32),
            ea=pack_col(ews[0]), eb=pack_col(ews[1]) if ews[1] else np.zeros((128, 1), np.float32),
            nreg=nreg,
        ))

    meta = dict(nt=nt, npad=npad, tpc=tpc, padw=padw, shard=shard,
                runs=runs, cpa=cpa, cpb=cpb, tile_map=tile_map,
                ca=int(cpa.sum()), cb=int(cpb.sum()))
    return per_core, meta, (grid_r, grid_s), cnts


def _launch1(meta, dt):
    import concourse.mybir as mybir
    import concourse.tile as tile
    from concourse import bacc

    f32 = mybir.dt.float32
    shard, padw = meta['shard'], meta['padw']
    ntile = shard // P
    nc = bacc.Bacc(None)
    xt = nc.declare_dram_parameter("xt", [P, shard], dt, isOutput=False)
    w = nc.declare_dram_parameter("w", [P, D], dt, isOutput=False)
    gr = nc.declare_dram_parameter("gr", [P, ntile, padw], dt, isOutput=False)
    gs = nc.declare_dram_parameter("gs", [P, ntile, padw], dt, isOutput=False)
    cntr = nc.declare_dram_parameter("cntr", [P, ntile], f32, isOutput=False)
    cnts = nc.declare_dram_parameter("cnts", [P, ntile], f32, isOutput=False)
    y = nc.declare_dram_parameter("y", [shard, D], dt, isOutput=True)

    with tile.TileContext(nc) as tc:
        with (
            tc.tile_pool(name="c", bufs=1) as cp,
            tc.tile_pool(name="g", bufs=1) as gp,
            tc.tile_pool(name="yo", bufs=1) as yp,
            tc.tile_pool(name="ps", bufs=4, space="PSUM") as pp,
        ):
            w_t = cp.tile([P, D], dt)
            nc.sync.dma_start(out=w_t[:], in_=w[:, :])
            xt_t = cp.tile([P, shard], dt)
            nc.sync.dma_start(out=xt_t[:], in_=xt[:, :])

            scale_t = cp.tile([P, ntile], f32, tag="sc")
            for nm, g, c in (("r", gr, cntr), ("s", gs, cnts)):
                g_t = gp.tile([P, ntile, padw], dt, tag="g")
                nc.sync.dma_start(out=g_t[:], in_=g[:, :, :])
                c_t = gp.tile([P, ntile], f32, tag="c" + nm)
                nc.sync.dma_start(out=c_t[:], in_=c[:, :])
                d_t = gp.tile([P, ntile], f32, tag="d" + nm)
                nc.vector.tensor_reduce(out=d_t[:], in_=g_t[:],
                                        axis=mybir.AxisListType.X,
                                        op=mybir.AluOpType.add)
                if nm == "r":
                    nc.vector.tensor_add(out=scale_t[:], in0=d_t[:], in1=c_t[:])
                else:
                    d2 = gp.tile([P, ntile], f32, tag="d2")
                    nc.vector.tensor_add(out=d2[:], in0=d_t[:], in1=c_t[:])
                    nc.vector.tensor_mul(out=scale_t[:], in0=scale_t[:], in1=d2[:])
            sq = cp.tile([P, ntile], f32, tag="sq")
            nc.scalar.activation(out=sq[:], in_=scale_t[:],
                                 func=mybir.ActivationFunctionType.Sqrt)
            nc.vector.reciprocal(out=scale_t[:], in_=sq[:])

            y_sb = yp.tile([P, ntile, D], dt)
            for j in range(ntile):
                ps = pp.tile([P, D], mybir.dt.float32)
                nc.tensor.matmul(out=ps[:], lhsT=xt_t[:, j * P:(j + 1) * P],
                                 rhs=w_t[:], start=True, stop=True)
                nc.vector.tensor_scalar(
                    out=y_sb[:, j, :], in0=ps[:],
                    scalar1=scale_t[:, j:j + 1], scalar2=None,
                    op0=mybir.AluOpType.mult)
            nc.sync.dma_start(
                out=y[:, :].rearrange("(j p) f -> p j f", p=P), in_=y_sb[:])
    nc.finalize()
    return nc


def _launch2(meta, ca, cb, la, lb, nreg_uniform, dt):
    import concourse.mybir as mybir
    import concourse.tile as tile
    from concourse import bacc

    tpc, npad = meta['tpc'], meta['npad']
    runs, cpa, cpb = meta['runs'], meta['cpa'], meta['cpb']
    shard_out = npad // NCORES

    f32 = mybir.dt.float32
    nc = bacc.Bacc(None, num_swdge_queues=NQ)
    ya = nc.declare_dram_parameter("ya", [SPLIT, D], dt, isOutput=False)
    yb = nc.declare_dram_parameter("yb", [npad - SPLIT, D], dt, isOutput=False)
    ia = nc.declare_dram_parameter("ia", [P, max(la // 16, 8)], mybir.dt.int16, isOutput=False)
    ib = nc.declare_dram_parameter("ib", [P, max(lb // 16, 8)], mybir.dt.int16, isOutput=False)
    ra = nc.declare_dram_parameter("ra", [P, max(ca, 1)], dt, isOutput=False)
    rb = nc.declare_dram_parameter("rb", [P, max(cb, 1)], dt, isOutput=False)
    ea = nc.declare_dram_parameter("ea", [P, max(ca, 1)], dt, isOutput=False)
    eb = nc.declare_dram_parameter("eb", [P, max(cb, 1)], dt, isOutput=False)
    iota = nc.declare_dram_parameter("iota", [P, P], dt, isOutput=False)
    yt = nc.declare_dram_parameter("yt", [tpc * P, D], dt, isOutput=False)
    o = nc.declare_dram_parameter("o", [shard_out, D], f32, isOutput=True)

    with tile.TileContext(nc) as tc:
        with (
            tc.tile_pool(name="c", bufs=1) as cp,
            tc.tile_pool(name="ga", bufs=2) as gap,
            tc.tile_pool(name="gb", bufs=2) as gbp,
            tc.tile_pool(name="s", bufs=3) as sp_,
            tc.tile_pool(name="oo", bufs=3) as op_,
            tc.tile_pool(name="ps", bufs=8, space="PSUM") as pp,
        ):
            iota_t = cp.tile([P, P], dt)
            nc.sync.dma_start(out=iota_t[:], in_=iota[:, :])
            ia_t = cp.tile([P, max(la // 16, 8)], mybir.dt.int16, tag="ia")
            nc.sync.dma_start(out=ia_t[:], in_=ia[:, :])
            ib_t = cp.tile([P, max(lb // 16, 8)], mybir.dt.int16, tag="ib")
            nc.sync.dma_start(out=ib_t[:], in_=ib[:, :])
            ra_t = cp.tile([P, max(ca, 1)], dt, tag="ra")
            nc.sync.dma_start(out=ra_t[:], in_=ra[:, :])
            rb_t = cp.tile([P, max(cb, 1)], dt, tag="rb")
            nc.sync.dma_start(out=rb_t[:], in_=rb[:, :])
            ea_t = cp.tile([P, max(ca, 1)], dt, tag="ea")
            nc.sync.dma_start(out=ea_t[:], in_=ea[:, :])
            eb_t = cp.tile([P, max(cb, 1)], dt, tag="eb")
            nc.sync.dma_start(out=eb_t[:], in_=eb[:, :])
            yt_t = cp.tile([P, tpc, D], dt, tag="yt")
            nc.sync.dma_start(
                out=yt_t[:], in_=yt[:, :].rearrange("(j p) f -> p j f", p=P))

            coff = [0, 0]    # running chunk offset per table
            psum = {}        # local tile -> psum tile
            started = {}
            qn = 0
            # group runs by their tile-group; runs list alternates A/B per group
            gi = 0
            while gi < len(runs):
                ab0, g, _ = runs[gi]
                # all runs for this group
                gruns = [runs[gi]]
                if gi + 1 < len(runs) and runs[gi + 1][1] == g:
                    gruns.append(runs[gi + 1])
                gi += len(gruns)

                for j in g:
                    psum[j] = pp.tile([P, D], mybir.dt.float32, name=f"psum{j}", tag="ps")
                    started[j] = False
                # total chunks per tile in this group (for stop flags)
                remain = {}
                for ab, gg, cps in gruns:
                    for j, nch in zip(gg, cps):
                        remain[j] = remain.get(j, 0) + nch
                for ab, gg, cps in gruns:
                    n = sum(cps)
                    if n == 0:
                        continue
                    tab, it, rt, et = ((ya, ia_t, ra_t, ea_t) if ab == 0
                                       else (yb, ib_t, rb_t, eb_t))
                    gp = gap if ab == 0 else gbp
                    c0 = coff[ab]
                    g_t = gp.tile([P, n, D], dt, tag="g")
                    # split large gathers to stay under the SWDGE ring cap
                    # and spread desc-gen across the 4 Q7 core pairs
                    nsub = _ceil(n, MAXCHUNK)
                    sub = _ceil(n, nsub)
                    s0 = 0
                    while s0 < n:
                        sn = min(sub, n - s0)
                        nc.gpsimd.dma_gather(
                            out_ap=g_t[:, s0:s0 + sn, :], in_ap=tab[:, :],
                            idxs_ap=it[:, (c0 + s0) * 8:(c0 + s0 + sn) * 8],
                            num_idxs=sn * P, num_idxs_reg=sn * P,
                            elem_size=D, single_packet=False, queue_num=qn)
                        qn = (qn + 1) % NQ
                        s0 += sn
                    s_t = sp_.tile([P, n, P], dt, tag="s")
                    nc.vector.tensor_tensor(
                        out=s_t[:],
                        in0=rt[:, c0:c0 + n, None].broadcast_to([P, n, P]),
                        in1=iota_t[:, None, :].broadcast_to([P, n, P]),
                        op=mybir.AluOpType.is_equal)
                    nc.vector.tensor_tensor(
                        out=s_t[:], in0=s_t[:],
                        in1=et[:, c0:c0 + n, None].broadcast_to([P, n, P]),
                        op=mybir.AluOpType.mult)
                    c = 0
                    for j, nch in zip(gg, cps):
                        for _ in range(nch):
                            remain[j] -= 1
                            nc.tensor.matmul(
                                out=psum[j][:], lhsT=s_t[:, c, :],
                                rhs=g_t[:, c, :],
                                start=not started[j], stop=remain[j] == 0)
                            started[j] = True
                            c += 1
                    coff[ab] += n
                # flush group psums: out = psum + y_tile (self-loop term)
                for j in g:
                    o_t = op_.tile([P, D], f32, tag="o")
                    nc.vector.tensor_tensor(
                        out=o_t[:], in0=psum[j][:], in1=yt_t[:, j, :],
                        op=mybir.AluOpType.add)
                    nc.sync.dma_start(out=o[j * P:(j + 1) * P, :], in_=o_t[:])
    nc.finalize()
    return nc


LAST_HW_NS = None


def _run(nc, in_maps):
    import os
    if os.environ.get("GCN_SIM"):
        from concourse.bass_interp import MultiCoreSim

        class R:
            pass

        sim = MultiCoreSim(nc, num_cores=len(in_maps))
        for k, core in sim.cores.items():
            for name, arr in in_maps[k].items():
                core.tensor(name)[:] = arr
        sim.simulate()
        r = R()
        r.results = [
            {n: sim.cores[k].tensor(n).copy()
             for n in ("y", "o") if _has_tensor(sim.cores[k], n)}
            for k in range(len(in_maps))]
        r.exec_time_ns = None
        return r
    from concourse.bass_utils import run_bass_kernel_spmd
    trace = bool(os.environ.get("GCN_TRACE"))
    last = None
    for attempt in range(3):
        try:
            return run_bass_kernel_spmd(
                nc, in_maps, list(range(len(in_maps))), trace=trace)
        except Exception as e:  # transient device faults: retry, drop trace
            last = e
            trace = False
            import time as _t
            _t.sleep(2.0)
    raise last


def _has_tensor(core, name):
    try:
        core.tensor(name)
        return True
    except Exception:
        return False


def kernel(nodes, senders, receivers, edges, W):
    global LAST_HW_NS
    import concourse.mybir as mybir
    import ml_dtypes

    bf16 = ml_dtypes.bfloat16
    dt = mybir.dt.bfloat16
    n_nodes = nodes.shape[0]
    per_core, meta, (grid_r, grid_s), (cnt_r, cnt_s) = _build(
        senders.astype(np.int64), receivers.astype(np.int64),
        edges.astype(np.float32), n_nodes, NCORES, SPLIT)
    npad, shard, padw, tpc = meta['npad'], meta['shard'], meta['padw'], meta['tpc']
    ntile = shard // P

    nodes_pad = np.zeros((npad, D), np.float32)
    nodes_pad[:n_nodes] = nodes
    nodesT = np.ascontiguousarray(nodes_pad.T).astype(bf16)

    def shard_grid(g, k):
        s = g[k * shard:(k + 1) * shard]                    # [shard, padw]
        return np.ascontiguousarray(
            s.reshape(ntile, P, padw).transpose(1, 0, 2)).astype(bf16)

    def shard_cnt(c, k):
        s = np.maximum(c[k * shard:(k + 1) * shard], 1.0)   # pad nodes: deg 1 -> scale 1
        return np.ascontiguousarray(s.reshape(ntile, P).T)  # [128, ntile]

    nc1 = _launch1(meta, dt)
    in1 = []
    for k in range(NCORES):
        in1.append(dict(
            xt=np.ascontiguousarray(nodesT[:, k * shard:(k + 1) * shard]),
            w=W.astype(np.float32).astype(bf16),
            gr=shard_grid(grid_r, k), gs=shard_grid(grid_s, k),
            cntr=shard_cnt(cnt_r, k), cnts=shard_cnt(cnt_s, k)))
    res1 = _run(nc1, in1)

    def as_bf16(a):
        a = np.asarray(a)
        if a.dtype == bf16:
            return a
        if a.dtype == np.uint16:
            return a.view(bf16)
        return a.astype(bf16)

    y_full = np.concatenate([as_bf16(res1.results[k]["y"])
                             for k in range(NCORES)], axis=0)

    ya = np.ascontiguousarray(y_full[:SPLIT])
    yb = np.ascontiguousarray(y_full[SPLIT:])

    # uniform num_idxs_reg per run: max over cores (extra idxs on other cores
    # are -1 pads; reg must equal valid count per core -> must be per-core!
    # dma_gather num_idxs_reg is a runtime scalar but baked per-program; use
    # per-core value via... it must be uniform -> use n*P with idx pads
    # replaced by 0-index (safe: ew=0 kills contribution).
    la = per_core[0]['ia'].shape[1] * 16
    lb = per_core[0]['ib'].shape[1] * 16
    ca = max(per_core[0]['ra'].shape[1], 1)
    cb = max(per_core[0]['rb'].shape[1], 1)
    nreg_uniform = []
    ridx = 0
    for ab, g, cps in meta['runs']:
        nreg_uniform.append(sum(cps) * P)
        ridx += 1

    iota_np = np.tile(np.arange(P, dtype=np.float32), (P, 1)).astype(bf16)
    tile_map = meta['tile_map']
    nc2 = _launch2(meta, ca, cb, la, lb, nreg_uniform, dt)
    in2 = []
    for k in range(NCORES):
        pc = per_core[k]
        ia = pc['ia'].copy()
        ib = pc['ib'].copy()
        ia[ia < 0] = 0   # pads -> row 0 (ew=0 kills it); uniform reg count
        ib[ib < 0] = 0
        yt_k = np.concatenate(
            [y_full[int(t) * P:(int(t) + 1) * P] for t in tile_map[k]], axis=0)
        in2.append(dict(ya=ya, yb=yb, ia=ia, ib=ib,
                        ra=pc['ra'].astype(bf16), rb=pc['rb'].astype(bf16),
                        ea=pc['ea'].astype(bf16), eb=pc['eb'].astype(bf16),
                        iota=iota_np, yt=np.ascontiguousarray(yt_k)))
    res2 = _run(nc2, in2)
    out = np.zeros((npad, D), np.float32)
    for k in range(NCORES):
        ok_ = res2.results[k]["o"]
        for j in range(tpc):
            t = int(tile_map[k, j])
            out[t * P:(t + 1) * P] = ok_[j * P:(j + 1) * P]
    t1 = res1.exec_time_ns or 0
    t2 = res2.exec_time_ns or 0
    LAST_HW_NS = (t1 + t2) if (t1 or t2) else None
    import os
    if os.environ.get("GCN_TRACE"):
        print(f"[kernel] launch1: {t1} ns, launch2: {t2} ns")
        try:
            import trace_util
            print("=== launch2 trace ===")
            trace_util.summarize(res2)
        except Exception as e:
            print("trace summary failed:", e)
    return np.ascontiguousarray(out[:n_nodes])

